# revision 14
# baseline (speedup 1.0000x reference)
"""LlamaAttention (B=2,S=2048,D=4096,H=32,KV=8) on 8 NeuronCores.

Sharding: tensor-parallel over heads. Core c owns Q heads 4c..4c+3 and KV
head c (GQA n_rep=4 means those Q heads all read exactly KV head c).
Per core: QKV projections (feature-major), RoPE, causal flash attention in
transposed-score layout (scores kept as S^T[k,q] so the softmax reduction
is a matmul-with-ones over the partition axis), per-(batch,q-block)
AllGather of the attention output (feature-major), column-sharded wo
matmul. Final output slices are concatenated host-side.

v2 over the original baseline:
- consolidated DMAs (multi-row rearranged transfers; ~10x fewer triggers)
- attention inner loop processes head pairs with the PV/denominator
  matmuls software-pipelined one k-block behind the score matmuls
- fast approximate reciprocal for the softmax denominator
- per-(b,qb) AllGathers pipelined with compute
"""
import sys
import math

sys.path.insert(0, "/opt/trn_rl_repo")

import numpy as np
from ml_dtypes import bfloat16

B, S, D = 2, 2048, 4096
H, KVH, HD = 32, 8, 128
T = B * S                      # 4096 tokens
NC = 8                         # cores
HPC = H // NC                  # 4 q heads / core
QC = HPC * HD                  # 512 q-proj cols / core
TBS = 512                      # token block size
NTB = T // TBS                 # 8 token blocks
NDC = D // 128                 # 32 contraction chunks
NQB = S // TBS                 # 4 q blocks per sequence
ROPE_THETA = 10000.0

_CACHE = {}


def _build_nc():
    from concourse import bacc, tile, mybir, bass_isa

    f32 = mybir.dt.float32
    bf16 = mybir.dt.bfloat16
    ADD = mybir.AluOpType.add
    MULT = mybir.AluOpType.mult
    EXP = mybir.ActivationFunctionType.Exp
    COPY = mybir.ActivationFunctionType.Copy

    nc = bacc.Bacc("TRN2", target_bir_lowering=False, debug=False,
                   enable_asserts=True, num_devices=NC)

    hiddenT = nc.dram_tensor("hiddenT", [D, T], bf16, kind="ExternalInput").ap()
    wq_d = nc.dram_tensor("wq", [D, QC], bf16, kind="ExternalInput").ap()
    wk_d = nc.dram_tensor("wk", [D, HD], bf16, kind="ExternalInput").ap()
    wv_d = nc.dram_tensor("wv", [D, HD], bf16, kind="ExternalInput").ap()
    wo_d = nc.dram_tensor("wo", [D, QC], bf16, kind="ExternalInput").ap()
    cos_d = nc.dram_tensor("cosT", [HD, S], bf16, kind="ExternalInput").ap()
    # signed sin: row d holds -sin for d<64, +sin for d>=64 (rotate_half sign)
    ssin_d = nc.dram_tensor("ssinT", [HD, S], bf16, kind="ExternalInput").ap()
    maskb_d = nc.dram_tensor("maskb", [S, TBS], bf16, kind="ExternalInput").ap()
    ident_d = nc.dram_tensor("ident", [128, 128], bf16, kind="ExternalInput").ap()
    outT = nc.dram_tensor("outT", [QC, T], f32, kind="ExternalOutput").ap()

    with tile.TileContext(nc) as tc:
        with tc.tile_pool(name="persist", bufs=1) as pp, \
             tc.tile_pool(name="dram", bufs=1, space="DRAM") as dram:
            # ---- persistent SBUF tensors (live across phases) ----
            qt_sb = pp.tile([128, HPC * T], bf16, tag="qt")     # QT per head
            kt_sb = pp.tile([128, T], bf16, tag="kt")           # KT
            v_sb = pp.tile([128, T], bf16, tag="v")             # V token-major
            maskb_sb = pp.tile([128, 16 * TBS], bf16, tag="maskb")
            ident_sb = pp.tile([128, 128], bf16, tag="ident")
            onesm_sb = pp.tile([128, 128], bf16, tag="onesm")

            nc.vector.memset(onesm_sb[:], 1.0)

            cc_in = [dram.tile([QC, TBS], bf16, tag=f"ccin{i}", name=f"ccin{i}")
                     for i in range(NTB)]
            cc_out = [dram.tile([D, TBS], bf16, addr_space="Shared",
                                tag=f"ccout{i}", name=f"ccout{i}")
                      for i in range(NTB)]

            # ================= Phase 1: QKV + RoPE =================
            with tc.tile_pool(name="w1", bufs=1) as wp, \
                 tc.tile_pool(name="hp", bufs=3) as hp, \
                 tc.tile_pool(name="rope", bufs=3) as rp, \
                 tc.tile_pool(name="qkvps", bufs=6, space="PSUM") as qkv_ps, \
                 tc.tile_pool(name="trps", bufs=2, space="PSUM") as tr_ps:
                wq_sb = wp.tile([128, NDC * QC], bf16, tag="wq")
                wk_sb = wp.tile([128, NDC * HD], bf16, tag="wk")
                wv_sb = wp.tile([128, NDC * HD], bf16, tag="wv")
                cos_sb = wp.tile([128, S], bf16, tag="cos")
                ssin_sb = wp.tile([128, S], bf16, tag="ssin")

                # consolidated weight loads: row-block g of wq covers
                # dc = 8g..8g+7 (1 MiB per transfer). Only the chunks the
                # first matmuls need are loaded up front; the rest are
                # interleaved with tb=0's activation loads so PE starts
                # (and stays) busy from the first microseconds.
                def _load_wq(g):
                    nc.sync.dma_start(
                        wq_sb[:, QC * 8 * g:QC * 8 * (g + 1)]
                        .rearrange("p (j c) -> p j c", j=8),
                        wq_d[1024 * g:1024 * (g + 1), :]
                        .rearrange("(j p) c -> p j c", p=128))

                def _load_wkv(g):
                    for w_sb, w_d in ((wk_sb, wk_d), (wv_sb, wv_d)):
                        nc.sync.dma_start(
                            w_sb[:, HD * 8 * g:HD * 8 * (g + 1)]
                            .rearrange("p (j c) -> p j c", j=8),
                            w_d[1024 * g:1024 * (g + 1), :]
                            .rearrange("(j p) c -> p j c", p=128))

                _load_wq(0)
                _load_wkv(0)

                def _load_trig():
                    nc.sync.dma_start(cos_sb[:], cos_d[:])
                    nc.sync.dma_start(ssin_sb[:], ssin_d[:])

                def _load_misc():
                    nc.sync.dma_start(ident_sb[:], ident_d[:])
                    # mask diag band (transposed per 512-block, row-stacked)
                    nc.sync.dma_start(
                        maskb_sb[:].rearrange("p (k q) -> p k q", k=16),
                        maskb_d[:, :].rearrange("(k p) q -> p k q", p=128))

                deferred = [lambda: _load_wkv(1), lambda: _load_wq(1),
                            lambda: _load_wkv(2), lambda: _load_wq(2),
                            lambda: _load_wkv(3), lambda: _load_wq(3),
                            _load_trig, _load_misc]

                for tb in range(NTB):
                    t0 = TBS * tb
                    ps_q = [qkv_ps.tile([128, TBS], f32, tag="qkv",
                                        name=f"psq{tb}_{i}") for i in range(HPC)]
                    ps_k = qkv_ps.tile([128, TBS], f32, tag="qkv")
                    ps_v = qkv_ps.tile([128, TBS], f32, tag="qkv")
                    for g in range(8):          # 4 dc chunks per transfer
                        ht4 = hp.tile([128, 4 * TBS], bf16, tag="ht")
                        nc.sync.dma_start(
                            ht4[:].rearrange("p (j t) -> p j t", j=4),
                            hiddenT[512 * g:512 * (g + 1), t0:t0 + TBS]
                            .rearrange("(j p) t -> p j t", p=128))
                        for j in range(4):
                            dc = 4 * g + j
                            ht = ht4[:, TBS * j:TBS * (j + 1)]
                            st, sp = dc == 0, dc == NDC - 1
                            for hl in range(HPC):
                                nc.tensor.matmul(
                                    ps_q[hl][:],
                                    wq_sb[:, QC * dc + HD * hl:QC * dc + HD * (hl + 1)],
                                    ht, start=st, stop=sp)
                            nc.tensor.matmul(ps_k[:], wk_sb[:, HD * dc:HD * (dc + 1)],
                                             ht, start=st, stop=sp)
                            nc.tensor.matmul(ps_v[:], wv_sb[:, HD * dc:HD * (dc + 1)],
                                             ht, start=st, stop=sp)
                        if tb == 0 and deferred:
                            deferred.pop(0)()

                    pos = TBS * (tb % NQB)   # position within sequence
                    cs = cos_sb[:, pos:pos + TBS]
                    ss = ssin_sb[:, pos:pos + TBS]
                    # RoPE for q heads and k
                    for idx in range(HPC + 1):
                        ps = ps_q[idx] if idx < HPC else ps_k
                        xf = rp.tile([128, TBS], f32, tag="xf")
                        nc.scalar.activation(xf[:], ps[:], COPY)
                        rot = rp.tile([128, TBS], f32, tag="rot")
                        nc.sync.dma_start(rot[0:64, :], xf[64:128, :])
                        nc.sync.dma_start(rot[64:128, :], xf[0:64, :])
                        t1 = rp.tile([128, TBS], f32, tag="t1")
                        nc.vector.tensor_tensor(t1[:], xf[:], cs, op=MULT)
                        t2 = rp.tile([128, TBS], f32, tag="t2")
                        nc.vector.tensor_tensor(t2[:], rot[:], ss, op=MULT)
                        if idx < HPC:
                            dst = qt_sb[:, T * idx + t0:T * idx + t0 + TBS]
                        else:
                            dst = kt_sb[:, t0:t0 + TBS]
                        nc.vector.tensor_tensor(dst, t1[:], t2[:], op=ADD)
                    # V: copy then transpose to token-major via DMA xbar
                    vtmp = rp.tile([128, TBS], bf16, tag="vtmp")
                    nc.scalar.activation(vtmp[:], ps_v[:], COPY)
                    for j in range(TBS // 128):
                        nc.sync.dma_start_transpose(
                            v_sb[:, t0 + 128 * j:t0 + 128 * (j + 1)],
                            vtmp[:, 128 * j:128 * (j + 1)])

            # ================= Phase 2: attention =================
            # Head pairs share one 2-bank score/exp tile (one EXP per pair
            # per k-block). Softmax denominators accumulate on VectorE in
            # bf16; one ones-matmul per head collapses partitions at the
            # end. PV/den work is software-pipelined one k-block behind the
            # score matmuls, and per-pair normalization tails are deferred
            # past the next pair's matmul burst so PE never waits on the
            # reciprocal. wo weights are DMA'd up front so the out
            # projection starts without a stall.
            with tc.tile_pool(name="w4", bufs=1) as wp4:
                wo_sb = wp4.tile([128, NDC * QC], bf16, tag="wo")
                for g in range(4):
                    nc.sync.dma_start(
                        wo_sb[:, QC * 8 * g:QC * 8 * (g + 1)]
                        .rearrange("p (j c) -> p j c", j=8),
                        wo_d[1024 * g:1024 * (g + 1), :]
                        .rearrange("(j p) c -> p j c", p=128))

                with tc.tile_pool(name="sps", bufs=2, space="PSUM") as sps, \
                     tc.tile_pool(name="ops", bufs=2, space="PSUM") as ops, \
                     tc.tile_pool(name="dps", bufs=2, space="PSUM") as dps, \
                     tc.tile_pool(name="ep", bufs=5) as ep, \
                     tc.tile_pool(name="accp", bufs=4) as accp, \
                     tc.tile_pool(name="osb", bufs=3) as osb, \
                     tc.tile_pool(name="rsb", bufs=3) as rsb, \
                     tc.tile_pool(name="atp", bufs=2) as atp:
                    at4s = {}
                    tail_state = {}

                    def kc_loop(b, qb, pr):
                        q0 = TBS * qb
                        nkc = 4 * (qb + 1)
                        heads = (2 * pr, 2 * pr + 1)
                        if pr == 0:
                            at4s[(b, qb)] = atp.tile(
                                [128, HPC * TBS], bf16, tag="at4",
                                name=f"at4_{b}_{qb}")
                        o_ps = {h: ops.tile([128, TBS], f32, tag="o",
                                            name=f"o{b}_{qb}_{h}")
                                for h in heads}
                        acc = {h: accp.tile([128, TBS], bf16, tag="acc",
                                            name=f"acc{b}_{qb}_{h}")
                               for h in heads}
                        pend = []       # PV matmuls run 2 k-blocks behind
                        def flush_pv(last):
                            pkc, pexp = pend.pop(0)
                            for i, h in enumerate(heads):
                                nc.tensor.matmul(
                                    o_ps[h][:],
                                    v_sb[:, S * b + 128 * pkc:S * b + 128 * (pkc + 1)],
                                    pexp[:, TBS * i:TBS * (i + 1)],
                                    start=pkc == 0, stop=last)
                        for kc in range(nkc):
                            spair = sps.tile([128, 2 * TBS], f32, tag="s",
                                             name=f"s{b}_{qb}_{pr}_{kc}")
                            for i, h in enumerate(heads):
                                qbase = T * h + S * b + q0
                                nc.tensor.matmul(
                                    spair[:, TBS * i:TBS * (i + 1)],
                                    kt_sb[:, S * b + 128 * kc:S * b + 128 * (kc + 1)],
                                    qt_sb[:, qbase:qbase + TBS],
                                    start=True, stop=True)
                            expair = ep.tile([128, 2 * TBS], bf16, tag="ex",
                                             name=f"ex{b}_{qb}_{pr}_{kc}")
                            nc.scalar.activation(expair[:], spair[:], EXP)
                            if 128 * (kc + 1) > TBS * qb:  # diagonal block
                                j = kc - 4 * qb
                                msl = maskb_sb[:, (4 * qb + j) * TBS:(4 * qb + j + 1) * TBS]
                                for i in range(2):
                                    nc.vector.tensor_tensor(
                                        expair[:, TBS * i:TBS * (i + 1)],
                                        expair[:, TBS * i:TBS * (i + 1)],
                                        msl, op=MULT)
                            for i, h in enumerate(heads):
                                esl = expair[:, TBS * i:TBS * (i + 1)]
                                if kc == 0:
                                    nc.vector.tensor_copy(acc[h][:], esl)
                                else:
                                    nc.vector.tensor_tensor(acc[h][:], acc[h][:],
                                                            esl, op=ADD)
                            pend.append((kc, expair))
                            if len(pend) > 2:
                                flush_pv(False)
                        while len(pend) > 1:
                            flush_pv(False)
                        flush_pv(True)
                        return heads, o_ps, acc

                    def tail_a(b, qb, pr, heads, o_ps, acc):
                        # free o banks fast; denominator = cross-partition
                        # all-reduce on GpSimd (result broadcast to all
                        # partitions), then fast reciprocal
                        o_sb, rec = {}, {}
                        for h in heads:
                            o_sb[h] = osb.tile([128, TBS], f32, tag="osb",
                                               name=f"osb{b}_{qb}_{h}")
                            nc.scalar.activation(o_sb[h][:], o_ps[h][:], COPY)
                        for h in heads:
                            dbc = dps.tile([128, TBS], f32, tag="den",
                                           name=f"dbc{b}_{qb}_{h}")
                            nc.tensor.matmul(dbc[:], onesm_sb[:], acc[h][:],
                                             start=True, stop=True)
                            rec[h] = rsb.tile([128, TBS], f32, tag="rec",
                                              name=f"rec{b}_{qb}_{h}")
                            with nc.allow_low_precision(reason="softmax denom"):
                                nc.vector.reciprocal_approx_fast(rec[h][:], dbc[:])
                        tail_state[(b, qb, pr)] = (heads, o_sb, rec)

                    def tail_b(b, qb, pr):
                        heads, o_sb, rec = tail_state.pop((b, qb, pr))
                        at4 = at4s[(b, qb)]
                        for h in heads:
                            nc.vector.tensor_tensor(
                                at4[:, TBS * h:TBS * (h + 1)],
                                o_sb[h][:], rec[h][:], op=MULT)
                        if pr == 1:
                            i = NQB * b + qb
                            nc.scalar.dma_start(
                                cc_in[i][:, :].rearrange("(h p) q -> p h q", p=128),
                                at4[:].rearrange("p (h q) -> p h q", h=HPC))
                            nc.gpsimd.collective_compute(
                                "AllGather", mybir.AluOpType.bypass,
                                replica_groups=[list(range(NC))],
                                ins=[cc_in[i].opt()], outs=[cc_out[i].opt()])

                    jobs = [(b, qb, pr) for b in range(B)
                            for qb in range(NQB) for pr in range(2)]
                    prev = None
                    for job in jobs:
                        args = kc_loop(*job)
                        if prev is not None:
                            tail_b(*prev)
                        tail_a(*job, *args)
                        prev = job
                    tail_b(*prev)

                # ================= Phase 3: out projection =================
                with tc.tile_pool(name="ap4", bufs=3) as ap4, \
                     tc.tile_pool(name="oc4", bufs=2) as oc4p, \
                     tc.tile_pool(name="outps", bufs=4, space="PSUM") as out_ps:
                    for i in range(NTB):
                        ps_o = [out_ps.tile([128, TBS], f32, tag="po",
                                            name=f"pso{i}_{nt}") for nt in range(4)]
                        for g in range(8):
                            at4w = ap4.tile([128, 4 * TBS], bf16, tag="a4")
                            nc.sync.dma_start(
                                at4w[:].rearrange("p (j q) -> p j q", j=4),
                                cc_out[i][512 * g:512 * (g + 1), :]
                                .rearrange("(j p) q -> p j q", p=128))
                            for j in range(4):
                                fc = 4 * g + j
                                st, sp = fc == 0, fc == NDC - 1
                                for nt in range(4):
                                    nc.tensor.matmul(
                                        ps_o[nt][:],
                                        wo_sb[:, QC * fc + 128 * nt:QC * fc + 128 * (nt + 1)],
                                        at4w[:, TBS * j:TBS * (j + 1)],
                                        start=st, stop=sp)
                        oc4 = oc4p.tile([128, 4 * TBS], f32, tag="oc")
                        for nt in range(4):
                            nc.scalar.activation(oc4[:, TBS * nt:TBS * (nt + 1)],
                                                 ps_o[nt][:], COPY)
                        nc.scalar.dma_start(
                            outT[:, TBS * i:TBS * (i + 1)]
                            .rearrange("(nt p) q -> p nt q", p=128),
                            oc4[:].rearrange("p (nt q) -> p nt q", nt=4))

    nc.compile()
    return nc


def _stage_inputs(hidden_states, wq, wk, wv, wo, attention_mask):
    hid = np.asarray(hidden_states, dtype=np.float32).reshape(T, D)
    hiddenT = np.ascontiguousarray(hid.T).astype(bfloat16)

    sc = 1.0 / math.sqrt(HD)
    mask = np.asarray(attention_mask, dtype=np.float32).reshape(S, S)
    # diag band, transposed: rows k in [512qb,512qb+512), cols q local.
    # Stored as a binary {0,1} multiplier applied to exp(scores).
    maskb = np.concatenate(
        [np.ascontiguousarray(mask[TBS * qb:TBS * (qb + 1),
                                   TBS * qb:TBS * (qb + 1)].T)
         for qb in range(NQB)], axis=0)
    maskb = (maskb == 0.0).astype(bfloat16)

    inv_freq = 1.0 / (ROPE_THETA ** (np.arange(0, HD, 2, dtype=np.float32) / HD))
    t = np.arange(S, dtype=np.float32)
    freqs = np.outer(t, inv_freq)
    emb = np.concatenate([freqs, freqs], axis=-1)        # [S, HD]
    cosT = np.ascontiguousarray(np.cos(emb).T).astype(bfloat16)   # [HD, S]
    sinT = np.ascontiguousarray(np.sin(emb).T)
    ssinT = sinT.copy()
    ssinT[:HD // 2] *= -1.0       # rotate_half sign: -sin for d<64
    ssinT = ssinT.astype(bfloat16)
    ident = np.eye(128, dtype=np.float32).astype(bfloat16)

    wq = np.asarray(wq, dtype=np.float32)
    wk = np.asarray(wk, dtype=np.float32)
    wv = np.asarray(wv, dtype=np.float32)
    wo = np.asarray(wo, dtype=np.float32)

    in_maps = []
    for c in range(NC):
        in_maps.append({
            "hiddenT": hiddenT,
            "wq": np.ascontiguousarray(wq[:, QC * c:QC * (c + 1)] * sc).astype(bfloat16),
            "wk": np.ascontiguousarray(wk[:, HD * c:HD * (c + 1)]).astype(bfloat16),
            "wv": np.ascontiguousarray(wv[:, HD * c:HD * (c + 1)]).astype(bfloat16),
            "wo": np.ascontiguousarray(wo[:, QC * c:QC * (c + 1)]).astype(bfloat16),
            "cosT": cosT, "ssinT": ssinT,
            "maskb": maskb, "ident": ident,
        })
    return in_maps


def kernel(hidden_states, wq, wk, wv, wo, attention_mask, _want_trace=False):
    from concourse import bass_utils

    if "nc" not in _CACHE:
        _CACHE["nc"] = _build_nc()
    nc = _CACHE["nc"]

    in_maps = _stage_inputs(hidden_states, wq, wk, wv, wo, attention_mask)
    res = bass_utils.run_bass_kernel_spmd(
        nc, in_maps, core_ids=list(range(NC)), trace=_want_trace)
    _CACHE["last_result"] = res

    outT_full = np.concatenate([res.results[c]["outT"] for c in range(NC)], axis=0)
    out = np.ascontiguousarray(outT_full.T).reshape(B, S, D).astype(np.float32)
    return out


# revision 16
# speedup vs baseline: 1.0646x; 1.0646x over previous
"""LlamaAttention (B=2,S=2048,D=4096,H=32,KV=8) on 8 NeuronCores.

Sharding: tensor-parallel over heads. Core c owns Q heads 4c..4c+3 and KV
head c (GQA n_rep=4 means those Q heads all read exactly KV head c).
Per core: QKV projections (feature-major), RoPE, causal flash attention in
transposed-score layout (scores kept as S^T[k,q] so the softmax reduction
is a matmul-with-ones over the partition axis), per-(batch,q-block)
AllGather of the attention output (feature-major), column-sharded wo
matmul. Final output slices are concatenated host-side.

v2 over the original baseline:
- consolidated DMAs (multi-row rearranged transfers; ~10x fewer triggers)
- attention inner loop processes head pairs with the PV/denominator
  matmuls software-pipelined one k-block behind the score matmuls
- fast approximate reciprocal for the softmax denominator
- per-(b,qb) AllGathers pipelined with compute
"""
import sys
import math

sys.path.insert(0, "/opt/trn_rl_repo")

import numpy as np
from ml_dtypes import bfloat16

B, S, D = 2, 2048, 4096
H, KVH, HD = 32, 8, 128
T = B * S                      # 4096 tokens
NC = 8                         # cores
HPC = H // NC                  # 4 q heads / core
QC = HPC * HD                  # 512 q-proj cols / core
TBS = 512                      # token block size
NTB = T // TBS                 # 8 token blocks
NDC = D // 128                 # 32 contraction chunks
NQB = S // TBS                 # 4 q blocks per sequence
ROPE_THETA = 10000.0

_CACHE = {}


def _build_nc():
    from concourse import bacc, tile, mybir, bass_isa

    f32 = mybir.dt.float32
    bf16 = mybir.dt.bfloat16
    ADD = mybir.AluOpType.add
    MULT = mybir.AluOpType.mult
    EXP = mybir.ActivationFunctionType.Exp
    COPY = mybir.ActivationFunctionType.Copy

    nc = bacc.Bacc("TRN2", target_bir_lowering=False, debug=False,
                   enable_asserts=True, num_devices=NC)

    hiddenT = nc.dram_tensor("hiddenT", [D, T], bf16, kind="ExternalInput").ap()
    wq_d = nc.dram_tensor("wq", [D, QC], bf16, kind="ExternalInput").ap()
    wk_d = nc.dram_tensor("wk", [D, HD], bf16, kind="ExternalInput").ap()
    wv_d = nc.dram_tensor("wv", [D, HD], bf16, kind="ExternalInput").ap()
    wo_d = nc.dram_tensor("wo", [D, QC], bf16, kind="ExternalInput").ap()
    cos_d = nc.dram_tensor("cosT", [HD, S], bf16, kind="ExternalInput").ap()
    # signed sin: row d holds -sin for d<64, +sin for d>=64 (rotate_half sign)
    ssin_d = nc.dram_tensor("ssinT", [HD, S], bf16, kind="ExternalInput").ap()
    maskb_d = nc.dram_tensor("maskb", [S, TBS], bf16, kind="ExternalInput").ap()
    ident_d = nc.dram_tensor("ident", [128, 128], bf16, kind="ExternalInput").ap()
    outT = nc.dram_tensor("outT", [QC, T], f32, kind="ExternalOutput").ap()

    with tile.TileContext(nc) as tc:
        with tc.tile_pool(name="persist", bufs=1) as pp, \
             tc.tile_pool(name="dram", bufs=1, space="DRAM") as dram:
            # ---- persistent SBUF tensors (live across phases) ----
            qt_sb = pp.tile([128, HPC * T], bf16, tag="qt")     # QT per head
            kt_sb = pp.tile([128, T], bf16, tag="kt")           # KT
            v_sb = pp.tile([128, T], bf16, tag="v")             # V token-major
            maskb_sb = pp.tile([128, 16 * TBS], bf16, tag="maskb")
            ident_sb = pp.tile([128, 128], bf16, tag="ident")
            onesm_sb = pp.tile([128, 128], bf16, tag="onesm")

            nc.vector.memset(onesm_sb[:], 1.0)

            cc_in = [dram.tile([QC, TBS], bf16, tag=f"ccin{i}", name=f"ccin{i}")
                     for i in range(NTB)]
            cc_out = [dram.tile([D, TBS], bf16, addr_space="Shared",
                                tag=f"ccout{i}", name=f"ccout{i}")
                      for i in range(NTB)]

            # ================= Phase 1: QKV + RoPE =================
            with tc.tile_pool(name="w1", bufs=1) as wp, \
                 tc.tile_pool(name="hp", bufs=3) as hp, \
                 tc.tile_pool(name="rope", bufs=3) as rp, \
                 tc.tile_pool(name="qkvps", bufs=6, space="PSUM") as qkv_ps, \
                 tc.tile_pool(name="trps", bufs=2, space="PSUM") as tr_ps:
                wq_sb = wp.tile([128, NDC * QC], bf16, tag="wq")
                wk_sb = wp.tile([128, NDC * HD], bf16, tag="wk")
                wv_sb = wp.tile([128, NDC * HD], bf16, tag="wv")
                cos_sb = wp.tile([128, S], bf16, tag="cos")
                ssin_sb = wp.tile([128, S], bf16, tag="ssin")

                # consolidated weight loads: row-block g of wq covers
                # dc = 8g..8g+7 (1 MiB per transfer). Only the chunks the
                # first matmuls need are loaded up front; the rest are
                # interleaved with tb=0's activation loads so PE starts
                # (and stays) busy from the first microseconds.
                def _load_wq(g, half=None):
                    lo, hi = (0, 8) if half is None else ((0, 4) if half == 0 else (4, 8))
                    nc.sync.dma_start(
                        wq_sb[:, QC * (8 * g + lo):QC * (8 * g + hi)]
                        .rearrange("p (j c) -> p j c", j=hi - lo),
                        wq_d[1024 * g + 128 * lo:1024 * g + 128 * hi, :]
                        .rearrange("(j p) c -> p j c", p=128))

                def _load_wkv(g):
                    for w_sb, w_d in ((wk_sb, wk_d), (wv_sb, wv_d)):
                        nc.sync.dma_start(
                            w_sb[:, HD * 8 * g:HD * 8 * (g + 1)]
                            .rearrange("p (j c) -> p j c", j=8),
                            w_d[1024 * g:1024 * (g + 1), :]
                            .rearrange("(j p) c -> p j c", p=128))

                _load_wq(0, half=0)
                _load_wkv(0)
                _load_wq(0, half=1)

                def _load_trig():
                    nc.sync.dma_start(cos_sb[:], cos_d[:])
                    nc.sync.dma_start(ssin_sb[:], ssin_d[:])

                def _load_misc():
                    nc.sync.dma_start(ident_sb[:], ident_d[:])
                    # mask diag band (transposed per 512-block, row-stacked)
                    nc.sync.dma_start(
                        maskb_sb[:].rearrange("p (k q) -> p k q", k=16),
                        maskb_d[:, :].rearrange("(k p) q -> p k q", p=128))

                deferred = [lambda: _load_wkv(1), lambda: _load_wq(1),
                            lambda: _load_wkv(2), lambda: _load_wq(2),
                            lambda: _load_wkv(3), lambda: _load_wq(3),
                            _load_trig, _load_misc]

                for tb in range(NTB):
                    t0 = TBS * tb
                    ps_q = [qkv_ps.tile([128, TBS], f32, tag="qkv",
                                        name=f"psq{tb}_{i}") for i in range(HPC)]
                    ps_k = qkv_ps.tile([128, TBS], f32, tag="qkv")
                    ps_v = qkv_ps.tile([128, TBS], f32, tag="qkv")
                    for g in range(8):          # 4 dc chunks per transfer
                        ht4 = hp.tile([128, 4 * TBS], bf16, tag="ht")
                        nc.sync.dma_start(
                            ht4[:].rearrange("p (j t) -> p j t", j=4),
                            hiddenT[512 * g:512 * (g + 1), t0:t0 + TBS]
                            .rearrange("(j p) t -> p j t", p=128))
                        for j in range(4):
                            dc = 4 * g + j
                            ht = ht4[:, TBS * j:TBS * (j + 1)]
                            st, sp = dc == 0, dc == NDC - 1
                            for hl in range(HPC):
                                nc.tensor.matmul(
                                    ps_q[hl][:],
                                    wq_sb[:, QC * dc + HD * hl:QC * dc + HD * (hl + 1)],
                                    ht, start=st, stop=sp)
                            nc.tensor.matmul(ps_k[:], wk_sb[:, HD * dc:HD * (dc + 1)],
                                             ht, start=st, stop=sp)
                            nc.tensor.matmul(ps_v[:], wv_sb[:, HD * dc:HD * (dc + 1)],
                                             ht, start=st, stop=sp)
                        if tb == 0 and deferred:
                            deferred.pop(0)()

                    pos = TBS * (tb % NQB)   # position within sequence
                    cs = cos_sb[:, pos:pos + TBS]
                    ss = ssin_sb[:, pos:pos + TBS]
                    # RoPE for q heads and k
                    for idx in range(HPC + 1):
                        ps = ps_q[idx] if idx < HPC else ps_k
                        xf = rp.tile([128, TBS], f32, tag="xf")
                        nc.scalar.activation(xf[:], ps[:], COPY)
                        rot = rp.tile([128, TBS], f32, tag="rot")
                        nc.sync.dma_start(rot[0:64, :], xf[64:128, :])
                        nc.sync.dma_start(rot[64:128, :], xf[0:64, :])
                        t1 = rp.tile([128, TBS], f32, tag="t1")
                        nc.vector.tensor_tensor(t1[:], xf[:], cs, op=MULT)
                        t2 = rp.tile([128, TBS], f32, tag="t2")
                        nc.vector.tensor_tensor(t2[:], rot[:], ss, op=MULT)
                        if idx < HPC:
                            dst = qt_sb[:, T * idx + t0:T * idx + t0 + TBS]
                        else:
                            dst = kt_sb[:, t0:t0 + TBS]
                        nc.vector.tensor_tensor(dst, t1[:], t2[:], op=ADD)
                    # V: copy then transpose to token-major
                    vtmp = rp.tile([128, TBS], bf16, tag="vtmp")
                    nc.scalar.activation(vtmp[:], ps_v[:], COPY)
                    for j in range(TBS // 128):
                        tp = tr_ps.tile([128, 128], bf16, tag="tr")
                        with nc.allow_low_precision(reason="PE transpose, no accum"):
                            nc.tensor.transpose(tp[:], vtmp[:, 128 * j:128 * (j + 1)],
                                                ident_sb[:])
                        nc.vector.tensor_copy(
                            v_sb[:, t0 + 128 * j:t0 + 128 * (j + 1)], tp[:])

            # ================= Phase 2: attention =================
            # Head pairs share one 2-bank score/exp tile (one EXP per pair
            # per k-block). Softmax denominators accumulate on VectorE in
            # bf16; one ones-matmul per head collapses partitions at the
            # end. PV/den work is software-pipelined one k-block behind the
            # score matmuls, and per-pair normalization tails are deferred
            # past the next pair's matmul burst so PE never waits on the
            # reciprocal. wo weights are DMA'd up front so the out
            # projection starts without a stall.
            with tc.tile_pool(name="w4", bufs=1) as wp4:
                wo_sb = wp4.tile([128, NDC * QC], bf16, tag="wo")
                for g in range(4):
                    nc.sync.dma_start(
                        wo_sb[:, QC * 8 * g:QC * 8 * (g + 1)]
                        .rearrange("p (j c) -> p j c", j=8),
                        wo_d[1024 * g:1024 * (g + 1), :]
                        .rearrange("(j p) c -> p j c", p=128))

                with tc.tile_pool(name="sps", bufs=2, space="PSUM") as sps, \
                     tc.tile_pool(name="ops", bufs=2, space="PSUM") as ops, \
                     tc.tile_pool(name="dps", bufs=2, space="PSUM") as dps, \
                     tc.tile_pool(name="ep", bufs=5) as ep, \
                     tc.tile_pool(name="accp", bufs=4) as accp, \
                     tc.tile_pool(name="osb", bufs=3) as osb, \
                     tc.tile_pool(name="rsb", bufs=3) as rsb, \
                     tc.tile_pool(name="atp", bufs=2) as atp:
                    at4s = {}
                    tail_state = {}

                    def kc_loop(b, qb, pr):
                        q0 = TBS * qb
                        nkc = 4 * (qb + 1)
                        heads = (2 * pr, 2 * pr + 1)
                        if pr == 0:
                            at4s[(b, qb)] = atp.tile(
                                [128, HPC * TBS], bf16, tag="at4",
                                name=f"at4_{b}_{qb}")
                        o_ps = {h: ops.tile([128, TBS], f32, tag="o",
                                            name=f"o{b}_{qb}_{h}")
                                for h in heads}
                        acc = {h: accp.tile([128, TBS], bf16, tag="acc",
                                            name=f"acc{b}_{qb}_{h}")
                               for h in heads}
                        pend = []       # PV matmuls run 2 k-blocks behind
                        def flush_pv(last):
                            pkc, pexp = pend.pop(0)
                            for i, h in enumerate(heads):
                                nc.tensor.matmul(
                                    o_ps[h][:],
                                    v_sb[:, S * b + 128 * pkc:S * b + 128 * (pkc + 1)],
                                    pexp[:, TBS * i:TBS * (i + 1)],
                                    start=pkc == 0, stop=last)
                        for kc in range(nkc):
                            spair = sps.tile([128, 2 * TBS], f32, tag="s",
                                             name=f"s{b}_{qb}_{pr}_{kc}")
                            for i, h in enumerate(heads):
                                qbase = T * h + S * b + q0
                                nc.tensor.matmul(
                                    spair[:, TBS * i:TBS * (i + 1)],
                                    kt_sb[:, S * b + 128 * kc:S * b + 128 * (kc + 1)],
                                    qt_sb[:, qbase:qbase + TBS],
                                    start=True, stop=True)
                            expair = ep.tile([128, 2 * TBS], bf16, tag="ex",
                                             name=f"ex{b}_{qb}_{pr}_{kc}")
                            nc.scalar.activation(expair[:], spair[:], EXP)
                            if 128 * (kc + 1) > TBS * qb:  # diagonal block
                                j = kc - 4 * qb
                                msl = maskb_sb[:, (4 * qb + j) * TBS:(4 * qb + j + 1) * TBS]
                                for i in range(2):
                                    nc.vector.tensor_tensor(
                                        expair[:, TBS * i:TBS * (i + 1)],
                                        expair[:, TBS * i:TBS * (i + 1)],
                                        msl, op=MULT)
                            for i, h in enumerate(heads):
                                esl = expair[:, TBS * i:TBS * (i + 1)]
                                if kc == 0:
                                    nc.vector.tensor_copy(acc[h][:], esl)
                                else:
                                    nc.vector.tensor_tensor(acc[h][:], acc[h][:],
                                                            esl, op=ADD)
                            pend.append((kc, expair))
                            if len(pend) > 2:
                                flush_pv(False)
                        while len(pend) > 1:
                            flush_pv(False)
                        flush_pv(True)
                        return heads, o_ps, acc

                    def tail_a(b, qb, pr, heads, o_ps, acc):
                        # free o banks fast; denominator = cross-partition
                        # all-reduce on GpSimd (result broadcast to all
                        # partitions), then fast reciprocal
                        o_sb, rec = {}, {}
                        for h in heads:
                            o_sb[h] = osb.tile([128, TBS], f32, tag="osb",
                                               name=f"osb{b}_{qb}_{h}")
                            nc.scalar.activation(o_sb[h][:], o_ps[h][:], COPY)
                        for h in heads:
                            dbc = dps.tile([128, TBS], f32, tag="den",
                                           name=f"dbc{b}_{qb}_{h}")
                            nc.tensor.matmul(dbc[:], onesm_sb[:], acc[h][:],
                                             start=True, stop=True)
                            rec[h] = rsb.tile([128, TBS], f32, tag="rec",
                                              name=f"rec{b}_{qb}_{h}")
                            with nc.allow_low_precision(reason="softmax denom"):
                                nc.vector.reciprocal_approx_fast(rec[h][:], dbc[:])
                        tail_state[(b, qb, pr)] = (heads, o_sb, rec)

                    def tail_b(b, qb, pr):
                        heads, o_sb, rec = tail_state.pop((b, qb, pr))
                        at4 = at4s[(b, qb)]
                        for h in heads:
                            nc.vector.tensor_tensor(
                                at4[:, TBS * h:TBS * (h + 1)],
                                o_sb[h][:], rec[h][:], op=MULT)
                        if pr == 1:
                            i = NQB * b + qb
                            nc.scalar.dma_start(
                                cc_in[i][:, :].rearrange("(h p) q -> p h q", p=128),
                                at4[:].rearrange("p (h q) -> p h q", h=HPC))
                            nc.gpsimd.collective_compute(
                                "AllGather", mybir.AluOpType.bypass,
                                replica_groups=[list(range(NC))],
                                ins=[cc_in[i].opt()], outs=[cc_out[i].opt()])

                    jobs = [(b, qb, pr) for b in range(B)
                            for qb in range(NQB) for pr in range(2)]
                    prev = None
                    for job in jobs:
                        args = kc_loop(*job)
                        if prev is not None:
                            tail_b(*prev)
                        tail_a(*job, *args)
                        prev = job
                    tail_b(*prev)

                # ================= Phase 3: out projection =================
                with tc.tile_pool(name="ap4", bufs=3) as ap4, \
                     tc.tile_pool(name="oc4", bufs=2) as oc4p, \
                     tc.tile_pool(name="outps", bufs=4, space="PSUM") as out_ps:
                    for i in range(NTB):
                        ps_o = [out_ps.tile([128, TBS], f32, tag="po",
                                            name=f"pso{i}_{nt}") for nt in range(4)]
                        for g in range(8):
                            at4w = ap4.tile([128, 4 * TBS], bf16, tag="a4")
                            nc.sync.dma_start(
                                at4w[:].rearrange("p (j q) -> p j q", j=4),
                                cc_out[i][512 * g:512 * (g + 1), :]
                                .rearrange("(j p) q -> p j q", p=128))
                            for j in range(4):
                                fc = 4 * g + j
                                st, sp = fc == 0, fc == NDC - 1
                                for nt in range(4):
                                    nc.tensor.matmul(
                                        ps_o[nt][:],
                                        wo_sb[:, QC * fc + 128 * nt:QC * fc + 128 * (nt + 1)],
                                        at4w[:, TBS * j:TBS * (j + 1)],
                                        start=st, stop=sp)
                        oc4 = oc4p.tile([128, 4 * TBS], f32, tag="oc")
                        for nt in range(4):
                            dst = oc4[:, TBS * nt:TBS * (nt + 1)]
                            if nt % 2 == 0:
                                nc.scalar.activation(dst, ps_o[nt][:], COPY)
                            else:
                                nc.vector.tensor_copy(dst, ps_o[nt][:])
                        nc.scalar.dma_start(
                            outT[:, TBS * i:TBS * (i + 1)]
                            .rearrange("(nt p) q -> p nt q", p=128),
                            oc4[:].rearrange("p (nt q) -> p nt q", nt=4))

    nc.compile()
    return nc


def _stage_inputs(hidden_states, wq, wk, wv, wo, attention_mask):
    hid = np.asarray(hidden_states, dtype=np.float32).reshape(T, D)
    hiddenT = np.ascontiguousarray(hid.T).astype(bfloat16)

    sc = 1.0 / math.sqrt(HD)
    mask = np.asarray(attention_mask, dtype=np.float32).reshape(S, S)
    # diag band, transposed: rows k in [512qb,512qb+512), cols q local.
    # Stored as a binary {0,1} multiplier applied to exp(scores).
    maskb = np.concatenate(
        [np.ascontiguousarray(mask[TBS * qb:TBS * (qb + 1),
                                   TBS * qb:TBS * (qb + 1)].T)
         for qb in range(NQB)], axis=0)
    maskb = (maskb == 0.0).astype(bfloat16)

    inv_freq = 1.0 / (ROPE_THETA ** (np.arange(0, HD, 2, dtype=np.float32) / HD))
    t = np.arange(S, dtype=np.float32)
    freqs = np.outer(t, inv_freq)
    emb = np.concatenate([freqs, freqs], axis=-1)        # [S, HD]
    cosT = np.ascontiguousarray(np.cos(emb).T).astype(bfloat16)   # [HD, S]
    sinT = np.ascontiguousarray(np.sin(emb).T)
    ssinT = sinT.copy()
    ssinT[:HD // 2] *= -1.0       # rotate_half sign: -sin for d<64
    ssinT = ssinT.astype(bfloat16)
    ident = np.eye(128, dtype=np.float32).astype(bfloat16)

    wq = np.asarray(wq, dtype=np.float32)
    wk = np.asarray(wk, dtype=np.float32)
    wv = np.asarray(wv, dtype=np.float32)
    wo = np.asarray(wo, dtype=np.float32)

    in_maps = []
    for c in range(NC):
        in_maps.append({
            "hiddenT": hiddenT,
            "wq": np.ascontiguousarray(wq[:, QC * c:QC * (c + 1)] * sc).astype(bfloat16),
            "wk": np.ascontiguousarray(wk[:, HD * c:HD * (c + 1)]).astype(bfloat16),
            "wv": np.ascontiguousarray(wv[:, HD * c:HD * (c + 1)]).astype(bfloat16),
            "wo": np.ascontiguousarray(wo[:, QC * c:QC * (c + 1)]).astype(bfloat16),
            "cosT": cosT, "ssinT": ssinT,
            "maskb": maskb, "ident": ident,
        })
    return in_maps


def kernel(hidden_states, wq, wk, wv, wo, attention_mask, _want_trace=False):
    from concourse import bass_utils

    if "nc" not in _CACHE:
        _CACHE["nc"] = _build_nc()
    nc = _CACHE["nc"]

    in_maps = _stage_inputs(hidden_states, wq, wk, wv, wo, attention_mask)
    res = bass_utils.run_bass_kernel_spmd(
        nc, in_maps, core_ids=list(range(NC)), trace=_want_trace)
    _CACHE["last_result"] = res

    outT_full = np.concatenate([res.results[c]["outT"] for c in range(NC)], axis=0)
    out = np.ascontiguousarray(outT_full.T).reshape(B, S, D).astype(np.float32)
    return out
